# revision 1
# baseline (speedup 1.0000x reference)
"""Trainium2 Bass kernel for NanodetLoss (nn_NanodetLoss_89343909692049).

Strategy
--------
Data-parallel over batch: core r handles images [8r, 8r+8), i.e. a
contiguous 32768-pixel slab of the flattened N = B*H*W axis.

The loss decomposes as
  qfl  = [ sum_{n,c} f(x_nc)  +  sum_{pos} lw*(pos_loss - f(x_at_lab)) ] / num_total
  bbox = 2    * sum_{pos} (1-giou)*wt
  dfl  = 1/16 * sum_{pos,k} dfl_k*wt
  wsum =        sum_{pos} wt
with f(x) = softplus(x)*sigmoid(x)^2 and wt = max_c sigmoid(x) at positives.
Everything except the dense f-sum and the per-pixel channel max only
matters at the ~2% positive anchors (labels < 80), so the host compacts the
positive rows (pure indexing; all math stays on device) and the device runs:
  dense: sigmoid / softplus (ACT), s*s (GPSIMD), (s^2*sp -> sum) (DVE TTR),
         running 5-way channel max of sigmoid (DVE), over cls flat [128,20480]
  pos:   on-chip gathers (GPSIMD indirect_copy) + PE transposes + masked
         reductions to pull sigma(x) at (pixel,label) and the channel max at
         positive pixels; then softmax/integral corners, GIoU, DFL and QFL
         positive terms on tiny [128, 8*k] tiles.
Per-core output is a [1,8] vector of partial sums; the host adds the 8
vectors and applies the scalar normalizations (pure epilogue).
"""

import sys

for _p in ("/opt/trn_rl_repo",):
    if _p not in sys.path:
        sys.path.insert(0, _p)

import numpy as np

import concourse.bass as bass
import concourse.mybir as mybir
from concourse.tile import TileContext
from concourse.vector_clock import ScopedClock
from concourse.bass_utils import run_bass_kernel_spmd

F32 = mybir.dt.float32
BF16 = mybir.dt.bfloat16
I32 = mybir.dt.int32
U16 = mybir.dt.uint16
AF = mybir.ActivationFunctionType
ALU = mybir.AluOpType
AX = mybir.AxisListType

# Problem geometry (fixed by the task spec).
B, C, R1 = 64, 80, 8
H = W = 64
HW = H * W                 # 4096
NCORES = 8
BPC = B // NCORES          # 8 batches per core
NPC = BPC * HW             # 32768 pixels per core
ROWF = BPC * C * HW // 128  # 20480 elements per SBUF row of the flat cls slab
CH = HW                    # dense chunk size (one channel-slice per row): 4096
NCH = ROWF // CH           # 5
POSCAP = 1024              # padded positive-slot capacity per core
T = POSCAP // 128          # 8 slot columns
REG_TOP = R1 - 1 - 0.1     # 6.9 bbox2distance clamp
EPS = 1e-6


class _SplitDrainTileContext(TileContext):
    """This container's walrus build rejects instructions carrying more than
    one sync-wait. Tile's wait assignment freely emits multi-waits, so after
    scheduling we hoist all but one wait of each instruction onto NOPs
    inserted right before it on the same engine (waiting earlier on the same
    engine is equivalent: every hoisted wait was already required there)."""

    def _drain_and_barrier(self, tick_clock, wait_clock):
        drain_inst = self.nc.sync.drain()
        wait_clock.add_sem_waits(
            drain_inst.ins, ScopedClock({None: tick_clock.global_clock})
        )
        waits = list(drain_inst.ins.sync_info.on_wait)
        if len(waits) > 1:
            drain_inst.ins.sync_info.on_wait = waits[:1]
            for w in waits[1:]:
                d2 = self.nc.sync.drain()
                d2.ins.sync_info = mybir.SyncInfo(on_wait=[w], on_update=[])
        self.nc.all_engine_barrier()
        assert self.sems is not None
        popped = self.nc._tile_sem_poison_stack.pop()
        assert popped is self._sem_poison
        self.nc.clear_and_free_semaphores(list(self.sems.allocated().values()))
        self.nc.all_engine_barrier()

    def schedule_and_allocate(self):
        ret = super().schedule_and_allocate()
        nc = self.nc
        for bb_name, bbw in list(nc.bb_map.items()):
            bb = bbw.bb
            insts = bb.instructions
            out = []
            changed = False
            for inst in insts:
                si = inst.sync_info
                if si is not None and si.on_wait and len(si.on_wait) > 1:
                    waits = list(si.on_wait)
                    for w in waits[:-1]:
                        nop = mybir.InstNoOp(
                            name=f"waitnop-{nc.next_id()}",
                            engine=inst.engine,
                            bass_nofuse=True,
                            sync_info=mybir.SyncInfo(on_wait=[w], on_update=[]),
                        )
                        nc.register_instruction(nop)
                        out.append(nop)
                    inst.sync_info = mybir.SyncInfo(
                        on_wait=[waits[-1]], on_update=list(si.on_update))
                    changed = True
                out.append(inst)
            if changed:
                bb.instructions = out
        return ret


def build_nc():
    nc = bass.Bass("TRN2", target_bir_lowering=False, debug=False,
                   num_devices=NCORES)

    cls_d = nc.dram_tensor("cls", [128, ROWF], F32, kind="ExternalInput")
    bbc_d = nc.dram_tensor("bbc", [128, T * 4 * R1], F32, kind="ExternalInput")
    tgt_d = nc.dram_tensor("tgt", [128, T * 4], F32, kind="ExternalInput")
    anc_d = nc.dram_tensor("anc", [128, T * 4], F32, kind="ExternalInput")
    wv_d = nc.dram_tensor("wv", [128, T], F32, kind="ExternalInput")
    lwv_d = nc.dram_tensor("lwv", [128, T], F32, kind="ExternalInput")
    qd_d = nc.dram_tensor("qd", [128, T], I32, kind="ExternalInput")
    bloc_d = nc.dram_tensor("bloc", [128, T], I32, kind="ExternalInput")
    idxd_d = nc.dram_tensor("idxd", [128, POSCAP // 16], U16, kind="ExternalInput")
    idxm_d = nc.dram_tensor("idxm", [128, POSCAP // 16], U16, kind="ExternalInput")
    strd_d = nc.dram_tensor("strd", [128, 1], F32, kind="ExternalInput")
    out_d = nc.dram_tensor("out", [1, 8], F32, kind="ExternalOutput")

    with _SplitDrainTileContext(nc) as tc:
        with (
            tc.tile_pool(name="const", bufs=1) as cpool,
            tc.tile_pool(name="xc", bufs=2) as xpool,
            tc.tile_pool(name="sfull", bufs=1) as spool,
            tc.tile_pool(name="dense", bufs=2) as dpool,
            tc.tile_pool(name="pos", bufs=1) as ppool,
            tc.tile_pool(name="ps", bufs=1, space="PSUM") as pspool,
        ):
            # ---------------- constants (gpsimd) ----------------
            ones_bf = cpool.tile([128, 128], BF16, tag="ones_bf", name="ones_bf")
            nc.gpsimd.memset(ones_bf[:], 1.0)
            id_bf = cpool.tile([128, 128], BF16, tag="id_bf", name="id_bf")
            nc.gpsimd.affine_select(id_bf[:], ones_bf[:], pattern=[[1, 128]],
                                    compare_op=ALU.is_equal, fill=0.0,
                                    base=0, channel_multiplier=-1)
            jio = cpool.tile([128, 1024], I32, tag="jio", name="jio")  # value = p_group j
            nc.gpsimd.iota(jio[:], pattern=[[0, T], [1, 8], [0, 16]],
                           base=0, channel_multiplier=0)
            pio = cpool.tile([128, 1024], I32, tag="pio", name="pio")  # value = src partition
            nc.gpsimd.iota(pio[:], pattern=[[0, T], [1, 128]],
                           base=0, channel_multiplier=0)
            jq8 = cpool.tile([128, T * 4 * R1], I32, tag="jq8", name="jq8")  # value = j in R1
            nc.gpsimd.iota(jq8[:], pattern=[[0, T], [0, 4], [1, R1]],
                           base=0, channel_multiplier=0)
            ones_col = cpool.tile([128, 1], F32, tag="ones_col", name="ones_col")
            nc.gpsimd.memset(ones_col[:], 1.0)

            # ---------------- small input loads ----------------
            def load(dram, shape, dtype, tag):
                t = cpool.tile(shape, dtype, tag=tag, name=tag)
                nc.sync.dma_start(out=t[:], in_=dram[:])
                return t

            bbc = load(bbc_d, [128, T * 4 * R1], F32, "bbc")
            tgt = load(tgt_d, [128, T * 4], F32, "tgt")
            anc = load(anc_d, [128, T * 4], F32, "anc")
            wv = load(wv_d, [128, T], F32, "wv")
            lwv = load(lwv_d, [128, T], F32, "lwv")
            qd = load(qd_d, [128, T], I32, "qd")
            bloc = load(bloc_d, [128, T], I32, "bloc")
            idxd = load(idxd_d, [128, POSCAP // 16], U16, "idxd")
            idxm = load(idxm_d, [128, POSCAP // 16], U16, "idxm")
            strd = load(strd_d, [128, 1], F32, "strd")

            # ---------------- dense pipeline ----------------
            sfull = spool.tile([128, ROWF], BF16, tag="sfull", name="sfull")
            xcs = []
            for k in range(NCH):
                xk = xpool.tile([128, CH], F32, tag="xchunk", name="xchunk")
                nc.sync.dma_start(out=xk[:], in_=cls_d[:, k * CH:(k + 1) * CH])
                xcs.append(xk)
            ssl = [sfull[:, k * CH:(k + 1) * CH] for k in range(NCH)]
            for k in range(NCH):
                nc.scalar.activation(ssl[k], xcs[k][:], AF.Sigmoid)

            # running max over the 5 channel-slices of each row (of sigmoid)
            mt = [dpool.tile([128, CH], BF16, tag="mping", name="mping", bufs=2) for _ in range(3)]
            mfin = cpool.tile([128, CH], BF16, tag="mfin", name="mfin")
            nc.vector.tensor_tensor(mt[0][:], ssl[0], ssl[1], ALU.max)
            nc.vector.tensor_tensor(mt[1][:], mt[0][:], ssl[2], ALU.max)
            nc.vector.tensor_tensor(mt[2][:], mt[1][:], ssl[3], ALU.max)
            nc.vector.tensor_tensor(mfin[:], mt[2][:], ssl[4], ALU.max)

            # softplus(x) = -ln(1 - sigmoid(x)); clamp keeps bf16 sigma
            # rounding at 1.0 out of the log.
            uks, s2s = [], []
            for k in range(NCH):
                uk = dpool.tile([128, CH], BF16, tag="uchunk", name="uchunk")
                nc.gpsimd.tensor_scalar_min(uk[:], ssl[k], 1.0 - 2.0 ** -8)
                uks.append(uk)
            spns = []
            for k in range(NCH):
                spk = dpool.tile([128, CH], BF16, tag="spchunk", name="spchunk")
                nc.scalar.activation(spk[:], uks[k][:], AF.Ln,
                                     scale=-1.0, bias=1.0)
                spns.append(spk)
            for k in range(NCH):
                s2k = dpool.tile([128, CH], BF16, tag="s2chunk", name="s2chunk")
                nc.gpsimd.tensor_tensor(s2k[:], ssl[k], ssl[k], ALU.mult)
                s2s.append(s2k)
            faccs = []
            for k in range(NCH):
                prk = dpool.tile([128, CH], BF16, tag="prchunk", name="prchunk",
                                 bufs=2)
                nc.vector.tensor_tensor(prk[:], s2s[k][:], spns[k][:], ALU.mult)
                fk = dpool.tile([128, 1], F32, tag="facc", name="facc", bufs=5)
                nc.vector.tensor_reduce(fk[:], prk[:], axis=AX.X, op=ALU.add)
                faccs.append(fk)
            fsum01 = ppool.tile([128, 1], F32, tag="fsum01", name="fsum01")
            fsum23 = ppool.tile([128, 1], F32, tag="fsum23", name="fsum23")
            fsum = ppool.tile([128, 1], F32, tag="fsum", name="fsum")
            nc.vector.tensor_tensor(fsum01[:], faccs[0][:], faccs[1][:], ALU.add)
            nc.vector.tensor_tensor(fsum23[:], faccs[2][:], faccs[3][:], ALU.add)
            nc.vector.tensor_tensor(fsum01[:], fsum01[:], fsum23[:], ALU.add)
            nc.vector.tensor_tensor(fsum[:], fsum01[:], faccs[4][:], ALU.add)
            nc.vector.tensor_scalar_mul(fsum[:], fsum[:], -1.0)

            # ---------------- gathers + transposes ----------------
            mg = ppool.tile([128, POSCAP], BF16, tag="mg", name="mg")
            nc.gpsimd.indirect_copy(mg[:], mfin[:], idxm[:], True)
            dgs = ppool.tile([128, POSCAP], BF16, tag="dgs", name="dgs")
            nc.gpsimd.indirect_copy(dgs[:], sfull[:], idxd[:], True)

            mgt = pspool.tile([128, POSCAP], BF16, tag="mgt", name="mgt")
            dgst = pspool.tile([128, POSCAP], BF16, tag="dgst", name="dgst")
            for t in range(T):
                sl = slice(t * 128, (t + 1) * 128)
                nc.tensor.transpose(mgt[:, sl], mg[:, sl], id_bf[:])
                nc.tensor.transpose(dgst[:, sl], dgs[:, sl], id_bf[:])

            # masks
            maskm = ppool.tile([128, POSCAP], BF16, tag="maskm", name="maskm")
            nc.vector.tensor_tensor(
                maskm[:].rearrange("p (t j l) -> p t j l", t=T, j=8, l=16),
                jio[:].rearrange("p (t j l) -> p t j l", t=T, j=8, l=16),
                bloc[:].unsqueeze(2).unsqueeze(3).broadcast_to((128, T, 8, 16)),
                ALU.is_equal)
            mask2 = ppool.tile([128, POSCAP], BF16, tag="mask2", name="mask2")
            nc.vector.tensor_tensor(
                mask2[:].rearrange("p (t l) -> p t l", t=T, l=128),
                pio[:].rearrange("p (t l) -> p t l", t=T, l=128),
                qd[:].unsqueeze(2).broadcast_to((128, T, 128)),
                ALU.is_equal)

            # wt = (max sigma over the pixel's 80 channels) at positive slots
            mm = ppool.tile([128, POSCAP], F32, tag="mm", name="mm")
            nc.vector.scalar_tensor_tensor(mm[:], mgt[:], 2.0, maskm[:],
                                           ALU.add, ALU.mult)
            wtm = ppool.tile([128, T], F32, tag="wtm", name="wtm")
            nc.vector.tensor_reduce(
                wtm[:], mm[:].rearrange("p (t l) -> p t l", t=T, l=128),
                axis=AX.X, op=ALU.max)
            wt = ppool.tile([128, T], F32, tag="wt", name="wt")
            nc.vector.tensor_scalar_add(wt[:], wtm[:], -2.0)
            wtv = ppool.tile([128, T], F32, tag="wtv", name="wtv")
            nc.vector.tensor_tensor(wtv[:], wt[:], wv[:], ALU.mult)

            # sigma(x) at (pixel, label)
            sxr = ppool.tile([128, POSCAP], F32, tag="sxr", name="sxr")
            nc.vector.tensor_tensor(sxr[:], dgst[:], mask2[:], ALU.mult)
            sxa = ppool.tile([128, T], F32, tag="sxa", name="sxa")
            nc.vector.tensor_reduce(
                sxa[:], sxr[:].rearrange("p (t l) -> p t l", t=T, l=128),
                axis=AX.X, op=ALU.add)

            # ---------------- positive branch ----------------
            def vtile(shape, tag):
                return ppool.tile(shape, F32, tag=tag, name=tag)

            def tt(out, a, b, op):
                nc.vector.tensor_tensor(out, a, b, op)

            # softmax + integral corners over the 4x8 bbox logit groups
            e = vtile([128, T * 32], "e")
            nc.scalar.activation(e[:], bbc[:], AF.Exp)
            S = vtile([128, T * 4], "S")
            nc.vector.tensor_reduce(
                S[:].rearrange("p (t k) -> p t k", t=T, k=4),
                e[:].rearrange("p (t k j) -> p t k j", t=T, k=4, j=R1),
                axis=AX.X, op=ALU.add)
            jf = vtile([128, T * 32], "jf")
            nc.vector.tensor_copy(jf[:], jq8[:])
            we = vtile([128, T * 32], "we")
            tt(we[:], e[:], jf[:], ALU.mult)
            wS = vtile([128, T * 4], "wS")
            nc.vector.tensor_reduce(
                wS[:].rearrange("p (t k) -> p t k", t=T, k=4),
                we[:].rearrange("p (t k j) -> p t k j", t=T, k=4, j=R1),
                axis=AX.X, op=ALU.add)
            rS = vtile([128, T * 4], "rS")
            nc.vector.reciprocal(rS[:], S[:])
            crn = vtile([128, T * 4], "crn")
            tt(crn[:], wS[:], rS[:], ALU.mult)

            # centers / normalized targets
            rstr = vtile([128, 1], "rstr")
            nc.vector.reciprocal(rstr[:], strd[:])
            rsh = vtile([128, 1], "rsh")
            nc.vector.tensor_scalar_mul(rsh[:], rstr[:], 0.5)
            anc3 = anc[:].rearrange("p (t c) -> p t c", t=T, c=4)
            ctr2 = vtile([128, T * 2], "ctr2")
            ctr2v = ctr2[:].rearrange("p (t c) -> p t c", t=T, c=2)
            tt(ctr2v, anc3[:, :, 0:2], anc3[:, :, 2:4], ALU.add)
            ctr = vtile([128, T * 2], "ctr")
            tt(ctr[:], ctr2[:], rsh[:].broadcast_to((128, T * 2)), ALU.mult)
            targ = vtile([128, T * 4], "targ")
            tt(targ[:], tgt[:], rstr[:].broadcast_to((128, T * 4)), ALU.mult)

            ctrv = ctr[:].rearrange("p (t c) -> p t c", t=T, c=2)
            crnv = crn[:].rearrange("p (t c) -> p t c", t=T, c=4)
            targv = targ[:].rearrange("p (t c) -> p t c", t=T, c=4)

            dec = vtile([128, T * 4], "dec")
            decv = dec[:].rearrange("p (t c) -> p t c", t=T, c=4)
            tt(decv[:, :, 0:2], ctrv, crnv[:, :, 0:2], ALU.subtract)
            tt(decv[:, :, 2:4], ctrv, crnv[:, :, 2:4], ALU.add)

            # aligned IoU + GIoU
            def sub2(tag, a, b):
                o = vtile([128, T * 2], tag)
                tt(o[:].rearrange("p (t c) -> p t c", t=T, c=2), a, b,
                   ALU.subtract)
                return o

            lt = vtile([128, T * 2], "lt")
            tt(lt[:].rearrange("p (t c) -> p t c", t=T, c=2),
               decv[:, :, 0:2], targv[:, :, 0:2], ALU.max)
            rb = vtile([128, T * 2], "rb")
            tt(rb[:].rearrange("p (t c) -> p t c", t=T, c=2),
               decv[:, :, 2:4], targv[:, :, 2:4], ALU.min)
            whr = vtile([128, T * 2], "whr")
            tt(whr[:], rb[:], lt[:], ALU.subtract)
            wh = vtile([128, T * 2], "wh")
            nc.vector.tensor_scalar_max(wh[:], whr[:], 0.0)
            whv = wh[:].rearrange("p (t c) -> p t c", t=T, c=2)
            ov = vtile([128, T], "ov")
            tt(ov[:].unsqueeze(2), whv[:, :, 0:1], whv[:, :, 1:2],
               ALU.mult)

            def area(tag, v):
                w_ = vtile([128, T * 2], tag + "wh")
                w_v = w_[:].rearrange("p (t c) -> p t c", t=T, c=2)
                tt(w_v, v[:, :, 2:4], v[:, :, 0:2], ALU.subtract)
                a_ = vtile([128, T], tag)
                tt(a_[:].unsqueeze(2), w_v[:, :, 0:1],
                   w_v[:, :, 1:2], ALU.mult)
                return a_

            ap_ = area("ap", decv)
            at_ = area("at", targv)
            un = vtile([128, T], "un")
            tt(un[:], ap_[:], at_[:], ALU.add)
            tt(un[:], un[:], ov[:], ALU.subtract)
            nc.vector.tensor_scalar_max(un[:], un[:], EPS)
            run_ = vtile([128, T], "run")
            nc.vector.reciprocal(run_[:], un[:])
            iou = vtile([128, T], "iou")
            tt(iou[:], ov[:], run_[:], ALU.mult)

            elt = vtile([128, T * 2], "elt")
            tt(elt[:].rearrange("p (t c) -> p t c", t=T, c=2),
               decv[:, :, 0:2], targv[:, :, 0:2], ALU.min)
            erb = vtile([128, T * 2], "erb")
            tt(erb[:].rearrange("p (t c) -> p t c", t=T, c=2),
               decv[:, :, 2:4], targv[:, :, 2:4], ALU.max)
            ewr = vtile([128, T * 2], "ewr")
            tt(ewr[:], erb[:], elt[:], ALU.subtract)
            ew = vtile([128, T * 2], "ew")
            nc.vector.tensor_scalar_max(ew[:], ewr[:], 0.0)
            ewv = ew[:].rearrange("p (t c) -> p t c", t=T, c=2)
            ea = vtile([128, T], "ea")
            tt(ea[:].unsqueeze(2), ewv[:, :, 0:1], ewv[:, :, 1:2],
               ALU.mult)
            nc.vector.tensor_scalar_max(ea[:], ea[:], EPS)
            rea = vtile([128, T], "rea")
            nc.vector.reciprocal(rea[:], ea[:])
            gd = vtile([128, T], "gd")
            tt(gd[:], ea[:], un[:], ALU.subtract)
            tt(gd[:], gd[:], rea[:], ALU.mult)
            giou = vtile([128, T], "giou")
            tt(giou[:], iou[:], gd[:], ALU.subtract)
            og = vtile([128, T], "og")
            nc.vector.tensor_scalar_mul(og[:], giou[:], -1.0)
            nc.vector.tensor_scalar_add(og[:], og[:], 1.0)
            lbs = vtile([128, T], "lbs")
            tt(lbs[:], og[:], wtv[:], ALU.mult)

            # DFL
            dist = vtile([128, T * 4], "dist")
            distv = dist[:].rearrange("p (t c) -> p t c", t=T, c=4)
            tt(distv[:, :, 0:2], ctrv, targv[:, :, 0:2], ALU.subtract)
            tt(distv[:, :, 2:4], targv[:, :, 2:4], ctrv, ALU.subtract)
            nc.vector.tensor_scalar_max(dist[:], dist[:], 0.0)
            nc.vector.tensor_scalar_min(dist[:], dist[:], REG_TOP)
            y = vtile([128, T * 32], "y")
            tt(y[:].rearrange("p (t k j) -> p t k j", t=T, k=4, j=R1),
               jf[:].rearrange("p (t k j) -> p t k j", t=T, k=4, j=R1),
               dist[:].rearrange("p (t k) -> p t k", t=T, k=4).unsqueeze(3)
                      .broadcast_to((128, T, 4, R1)),
               ALU.subtract)
            yn = vtile([128, T * 32], "yn")
            nc.vector.tensor_scalar_mul(yn[:], y[:], -1.0)
            ya = vtile([128, T * 32], "ya")
            tt(ya[:], y[:], yn[:], ALU.max)
            tent = vtile([128, T * 32], "tent")
            nc.vector.tensor_scalar_mul(tent[:], ya[:], -1.0)
            nc.vector.tensor_scalar_add(tent[:], tent[:], 1.0)
            nc.vector.tensor_scalar_max(tent[:], tent[:], 0.0)
            xt = vtile([128, T * 32], "xt")
            tt(xt[:], bbc[:], tent[:], ALU.mult)
            xts = vtile([128, T * 4], "xts")
            nc.vector.tensor_reduce(
                xts[:].rearrange("p (t k) -> p t k", t=T, k=4),
                xt[:].rearrange("p (t k j) -> p t k j", t=T, k=4, j=R1),
                axis=AX.X, op=ALU.add)
            lse = vtile([128, T * 4], "lse")
            nc.scalar.activation(lse[:], S[:], AF.Ln)
            dfk = vtile([128, T * 4], "dfk")
            tt(dfk[:], lse[:], xts[:], ALU.subtract)
            dfr = vtile([128, T], "dfr")
            nc.vector.tensor_reduce(
                dfr[:], dfk[:].rearrange("p (t k) -> p t k", t=T, k=4),
                axis=AX.X, op=ALU.add)
            dfs = vtile([128, T], "dfs")
            tt(dfs[:], dfr[:], wtv[:], ALU.mult)

            # QFL positive corrections
            sxl = vtile([128, T], "sxl")
            nc.vector.tensor_scalar_max(sxl[:], sxa[:], 1e-7)
            u2 = vtile([128, T], "u2")
            nc.vector.tensor_scalar_mul(u2[:], sxl[:], -1.0)
            nc.vector.tensor_scalar_add(u2[:], u2[:], 1.0)
            nc.vector.tensor_scalar_max(u2[:], u2[:], 1e-7)
            lns = vtile([128, T], "lns")
            nc.scalar.activation(lns[:], sxl[:], AF.Ln)
            ln1m = vtile([128, T], "ln1m")
            nc.scalar.activation(ln1m[:], u2[:], AF.Ln)
            xa = vtile([128, T], "xa")
            tt(xa[:], lns[:], ln1m[:], ALU.subtract)
            spxa = vtile([128, T], "spxa")
            nc.vector.tensor_scalar_mul(spxa[:], ln1m[:], -1.0)
            sxa2 = vtile([128, T], "sxa2")
            tt(sxa2[:], sxl[:], sxl[:], ALU.mult)
            fxa = vtile([128, T], "fxa")
            tt(fxa[:], sxa2[:], spxa[:], ALU.mult)
            xsc = vtile([128, T], "xsc")
            tt(xsc[:], xa[:], iou[:], ALU.mult)
            bce = vtile([128, T], "bce")
            tt(bce[:], spxa[:], xsc[:], ALU.subtract)
            sf = vtile([128, T], "sf")
            tt(sf[:], iou[:], sxl[:], ALU.subtract)
            sf2 = vtile([128, T], "sf2")
            tt(sf2[:], sf[:], sf[:], ALU.mult)
            pl = vtile([128, T], "pl")
            tt(pl[:], bce[:], sf2[:], ALU.mult)
            qc = vtile([128, T], "qc")
            tt(qc[:], pl[:], fxa[:], ALU.subtract)
            tt(qc[:], qc[:], lwv[:], ALU.mult)

            # ---------------- final partials ----------------
            def redcol(tag, src):
                o = vtile([128, 1], tag)
                nc.vector.tensor_reduce(o[:], src[:], axis=AX.X, op=ALU.add)
                return o

            qa = redcol("qa", qc)
            lba = redcol("lba", lbs)
            dfa = redcol("dfa", dfs)
            wta = redcol("wta", wtv)

            fin = vtile([128, 8], "fin")
            nc.vector.memset(fin[:], 0.0)
            nc.vector.tensor_copy(fin[:, 0:1], fsum[:])
            nc.vector.tensor_copy(fin[:, 1:2], qa[:])
            nc.vector.tensor_copy(fin[:, 2:3], lba[:])
            nc.vector.tensor_copy(fin[:, 3:4], dfa[:])
            nc.vector.tensor_copy(fin[:, 4:5], wta[:])

            outp = pspool.tile([1, 8], F32, tag="outp", name="outp")
            nc.tensor.matmul(out=outp[:], lhsT=ones_col[:], rhs=fin[:],
                             start=True, stop=True)
            outs = vtile([1, 8], "outs")
            nc.vector.tensor_copy(outs[:], outp[:])
            nc.sync.dma_start(out=out_d[:], in_=outs[:])

    return nc


_NC = None


def _get_nc():
    global _NC
    if _NC is None:
        _NC = build_nc()
    return _NC


def make_in_maps(anchors, cls_score, bbox_pred, label_weights, bbox_targets,
                 labels):
    """Host-side sharding + positive-row compaction (pure indexing)."""
    cls_score = np.ascontiguousarray(cls_score, np.float32)
    bbox_pred = np.ascontiguousarray(bbox_pred, np.float32)
    labels = np.asarray(labels, np.int32)
    label_weights = np.asarray(label_weights, np.float32)
    bbox_targets = np.asarray(bbox_targets, np.float32)
    anchors = np.asarray(anchors, np.float32)

    def fold(v):  # [POSCAP, k] -> [128, T*k] with slot i = p + 128*t
        k = v.shape[1] if v.ndim > 1 else 1
        return np.ascontiguousarray(
            v.reshape(T, 128, k).transpose(1, 0, 2).reshape(128, T * k))

    def wrap16(idx):  # uint16 wrapped index layout, replicated per 16-group
        w = idx.reshape(POSCAP // 16, 16).T.astype(np.uint16)  # [16, 64]
        return np.ascontiguousarray(np.tile(w, (8, 1)))

    in_maps = []
    for r in range(NCORES):
        base = r * NPC
        lab = labels[base:base + NPC]
        pos = np.nonzero(lab < C)[0]
        npos = len(pos)
        assert npos <= POSCAP, f"positive count {npos} exceeds cap {POSCAP}"
        idx = np.zeros(POSCAP, np.int64)
        idx[:npos] = pos
        valid = np.zeros(POSCAP, np.float32)
        valid[:npos] = 1.0
        b_loc = idx // HW
        hw = idx % HW
        labp = np.where(valid > 0, lab[idx], 0).astype(np.int64)
        gidx = base + idx

        bbc = bbox_pred.reshape(B, 32, HW)[r * BPC + b_loc, :, hw]  # [P, 32]
        tgt = bbox_targets[gidx]                                    # [P, 4]
        anc = anchors[gidx]                                         # [P, 4]
        lwv = label_weights[gidx] * valid
        qdv = np.where(valid > 0, 16 * b_loc + labp // 5, -1).astype(np.int32)
        blocv = np.where(valid > 0, b_loc, -1).astype(np.int32)
        idxd = hw + (labp % 5) * HW
        idxm = hw

        in_maps.append({
            "cls": cls_score[r * BPC:(r + 1) * BPC].reshape(128, ROWF),
            "bbc": fold(bbc),
            "tgt": fold(tgt),
            "anc": fold(anc),
            "wv": fold(valid[:, None]),
            "lwv": fold(lwv[:, None]),
            "qd": fold(qdv[:, None].astype(np.int32)),
            "bloc": fold(blocv[:, None].astype(np.int32)),
            "idxd": wrap16(idxd),
            "idxm": wrap16(idxm),
            "strd": np.zeros((128, 1), np.float32),  # patched by caller
        })
    return in_maps


def combine(results, num_total_samples):
    tot = np.zeros(8, np.float64)
    for r in results:
        tot += r["out"].reshape(8).astype(np.float64)
    qfl = (tot[0] + tot[1]) / float(num_total_samples)
    bbox = 2.0 * tot[2]
    dfl = tot[3] * 0.0625
    wsum = tot[4]
    return np.array([qfl, bbox, dfl, wsum], np.float32)


def kernel(anchors, cls_score, bbox_pred, label_weights, bbox_targets,
           labels, num_total_samples, stride):
    in_maps = make_in_maps(anchors, cls_score, bbox_pred, label_weights,
                           bbox_targets, labels)
    for m in in_maps:
        m["strd"] = np.full((128, 1), float(stride), np.float32)
    nc = _get_nc()
    res = run_bass_kernel_spmd(nc, in_maps, list(range(NCORES)))
    return combine(res.results, num_total_samples)


if __name__ == "__main__":
    pass



# revision 13
# speedup vs baseline: 2.1985x; 2.1985x over previous
"""Trainium2 Bass kernel for NanodetLoss (nn_NanodetLoss_89343909692049).

Strategy (v2)
-------------
Data-parallel over batch: core r handles images [8r, 8r+8), i.e. a
contiguous 32768-pixel slab of the flattened N = B*H*W axis.

The loss decomposes as
  qfl  = [ sum_{n,c} f(x_nc)  +  sum_{pos} lw*(pos_loss - f(x_at_lab)) ] / num_total
  bbox = 2    * sum_{pos} (1-giou)*wt
  dfl  = 1/16 * sum_{pos,k} dfl_k*wt
  wsum =        sum_{pos} wt
with f(x) = softplus(x)*sigmoid(x)^2 and wt = max_c sigmoid(x) at positives.

Dense pipeline per core (cls slab [128, 20480] fp16 = 5 channel-slices of
4096 pixels, streamed in 2048-wide halves):
  Act: s = Sigmoid(x) fp16   (all 10 halves, one table set)
  DVE: q = s*s               (runs inside the sigmoid window)
  Act: sp = -Ln((1+2^-24) - s) = softplus(x)   (one table switch)
  DVE: tensor_tensor_reduce(sp*q -> running row-sum accum)
All positive-anchor work (~2% of pixels, host-compacted by pure indexing)
runs on [128, 1024-slot] tiles: 5 gpsimd gathers (one per channel-slice)
at the positive pixel offsets, PE transposes into PSUM, then host-built
select/group masks give max_c sigma and sigma at the label via small
masked reductions. softmax/integral corners, GIoU, DFL and QFL positive
terms run on tiny [128, 8*k] tiles; exp(x) for the softmax is
sigma(x)/(1-sigma(x)) so only two activation table sets load in total.
Per-core output is a [1,8] vector of partial sums; the host adds the 8
vectors and applies the scalar normalizations (pure epilogue).
"""

import sys

for _p in ("/opt/trn_rl_repo",):
    if _p not in sys.path:
        sys.path.insert(0, _p)

import numpy as np

import concourse.bass as bass
import concourse.mybir as mybir
from concourse.tile import TileContext
from concourse.vector_clock import ScopedClock
from concourse.bass_utils import run_bass_kernel_spmd

F32 = mybir.dt.float32
F16 = mybir.dt.float16
I32 = mybir.dt.int32
U16 = mybir.dt.uint16
AF = mybir.ActivationFunctionType
ALU = mybir.AluOpType
AX = mybir.AxisListType

# Problem geometry (fixed by the task spec).
B, C, R1 = 64, 80, 8
H = W = 64
HW = H * W                 # 4096
NCORES = 8
BPC = B // NCORES          # 8 batches per core
NPC = BPC * HW             # 32768 pixels per core
ROWF = BPC * C * HW // 128  # 20480 elements per SBUF row of the flat cls slab
CH = HW                    # channel-slice size (one channel per row): 4096
NCH = ROWF // CH           # 5
HCH = CH // 2              # 2048-wide streaming halves
POSCAP = 1024              # padded positive-slot capacity per core
T = POSCAP // 128          # 8 slot columns
REG_TOP = R1 - 1 - 0.1     # 6.9 bbox2distance clamp
EPS = 1e-6
LNB = 1.0 + 2.0 ** -23     # softplus ln bias; guards ln(0) at sigma==1


class _SplitDrainTileContext(TileContext):
    """This container's walrus build rejects instructions carrying more than
    one sync-wait. Tile's wait assignment freely emits multi-waits, so after
    scheduling we hoist all but one wait of each instruction onto NOPs
    inserted right before it on the same engine (waiting earlier on the same
    engine is equivalent: every hoisted wait was already required there)."""

    def _drain_and_barrier(self, tick_clock, wait_clock):
        drain_inst = self.nc.sync.drain()
        wait_clock.add_sem_waits(
            drain_inst.ins, ScopedClock({None: tick_clock.global_clock})
        )
        waits = list(drain_inst.ins.sync_info.on_wait)
        if len(waits) > 1:
            drain_inst.ins.sync_info.on_wait = waits[:1]
            for w in waits[1:]:
                d2 = self.nc.sync.drain()
                d2.ins.sync_info = mybir.SyncInfo(on_wait=[w], on_update=[])
        self.nc.all_engine_barrier()
        assert self.sems is not None
        popped = self.nc._tile_sem_poison_stack.pop()
        assert popped is self._sem_poison
        self.nc.clear_and_free_semaphores(list(self.sems.allocated().values()))
        self.nc.all_engine_barrier()

    def schedule_and_allocate(self):
        ret = super().schedule_and_allocate()
        nc = self.nc
        for bb_name, bbw in list(nc.bb_map.items()):
            bb = bbw.bb
            insts = bb.instructions
            out = []
            changed = False
            for inst in insts:
                si = inst.sync_info
                if si is not None and si.on_wait and len(si.on_wait) > 1:
                    waits = list(si.on_wait)
                    for w in waits[:-1]:
                        nop = mybir.InstNoOp(
                            name=f"waitnop-{nc.next_id()}",
                            engine=inst.engine,
                            bass_nofuse=True,
                            sync_info=mybir.SyncInfo(on_wait=[w], on_update=[]),
                        )
                        nc.register_instruction(nop)
                        out.append(nop)
                    inst.sync_info = mybir.SyncInfo(
                        on_wait=[waits[-1]], on_update=list(si.on_update))
                    changed = True
                out.append(inst)
            if changed:
                bb.instructions = out
        return ret


def build_nc():
    nc = bass.Bass("TRN2", target_bir_lowering=False, debug=False,
                   num_devices=NCORES)

    cls_d = nc.dram_tensor("cls", [128, ROWF], F16, kind="ExternalInput")
    bbc_d = nc.dram_tensor("bbc", [128, T * 4 * R1], F32, kind="ExternalInput")
    tgt_d = nc.dram_tensor("tgt", [128, T * 4], F32, kind="ExternalInput")
    anc_d = nc.dram_tensor("anc", [128, T * 4], F32, kind="ExternalInput")
    wv_d = nc.dram_tensor("wv", [128, T], F32, kind="ExternalInput")
    lwv_d = nc.dram_tensor("lwv", [128, T], F32, kind="ExternalInput")
    jf_d = nc.dram_tensor("jfv", [128, T * 4 * R1], F32, kind="ExternalInput")
    maskm_d = nc.dram_tensor("maskm", [128, POSCAP], F16, kind="ExternalInput")
    mks_d = nc.dram_tensor("mks", [128, NCH * POSCAP], F16,
                           kind="ExternalInput")
    idxm_d = nc.dram_tensor("idxm", [128, POSCAP // 16], U16,
                            kind="ExternalInput")
    strd_d = nc.dram_tensor("strd", [128, 1], F32, kind="ExternalInput")
    out_d = nc.dram_tensor("out", [1, 8], F32, kind="ExternalOutput")

    with _SplitDrainTileContext(nc) as tc:
        with (
            tc.tile_pool(name="const", bufs=1) as cpool,
            tc.tile_pool(name="xc", bufs=3) as xpool,
            tc.tile_pool(name="sg", bufs=5) as spool,
            tc.tile_pool(name="dense", bufs=2) as dpool,
            tc.tile_pool(name="pos", bufs=1) as ppool,
            tc.tile_pool(name="ps", bufs=1, space="PSUM") as pspool,
        ):
            def vtile(shape, tag):
                return ppool.tile(shape, F32, tag=tag, name=tag)

            def tt(out, a, b, op):
                nc.vector.tensor_tensor(out, a, b, op)

            # ---------------- constants (gpsimd) ----------------
            ones_col = cpool.tile([128, 1], F32, tag="ones_col", name="ones_col")
            nc.gpsimd.memset(ones_col[:], 1.0)
            ones16 = cpool.tile([128, 128], F16, tag="ones16", name="ones16")
            nc.gpsimd.memset(ones16[:], 1.0)
            id16 = cpool.tile([128, 128], F16, tag="id16", name="id16")
            nc.gpsimd.affine_select(id16[:], ones16[:], pattern=[[1, 128]],
                                    compare_op=ALU.is_equal, fill=0.0,
                                    base=0, channel_multiplier=-1)
            lnb = cpool.tile([128, 1], F32, tag="lnb", name="lnb")
            nc.gpsimd.memset(lnb[:], LNB)
            ones16c = cpool.tile([128, 1], F16, tag="ones16c", name="ones16c")
            nc.gpsimd.memset(ones16c[:], 1.0)

            # ---------------- small input loads ----------------
            def load(dram, shape, dtype, tag):
                t = cpool.tile(shape, dtype, tag=tag, name=tag)
                nc.sync.dma_start(out=t[:], in_=dram[:])
                return t

            bbc = load(bbc_d, [128, T * 4 * R1], F32, "bbc")
            tgt = load(tgt_d, [128, T * 4], F32, "tgt")
            anc = load(anc_d, [128, T * 4], F32, "anc")
            wv = load(wv_d, [128, T], F32, "wv")
            lwv = load(lwv_d, [128, T], F32, "lwv")
            jf = load(jf_d, [128, T * 4 * R1], F32, "jf")
            idxm = load(idxm_d, [128, POSCAP // 16], U16, "idxm")
            strd = load(strd_d, [128, 1], F32, "strd")

            # ---------------- Act phase 0: table prefetch + exp ---------
            dummy = cpool.tile([128, 1], F32, tag="dummy", name="dummy")
            nc.scalar.activation(dummy[:], ones_col[:], AF.Sigmoid)
            # exp(x) for the bbox softmax via sigma/(1-sigma).
            esg = ppool.tile([128, T * 32], F32, tag="esg", name="esg")
            nc.scalar.activation(esg[:], bbc[:], AF.Sigmoid)

            # ---------------- dense stream: sigmoid + q = s^2 -----------
            sgs, qts, gts = [], [], []
            maskm = None
            mks = None
            for k in range(NCH):
                xk = xpool.tile([128, CH], F16, tag="xchunk", name="xchunk")
                for h in range(2):
                    sl = slice(h * HCH, (h + 1) * HCH)
                    nc.sync.dma_start(out=xk[:, sl],
                                      in_=cls_d[:, k * CH + h * HCH:
                                                k * CH + (h + 1) * HCH])
                if k == 1:
                    # positive-branch masks ride the DMA stream here: late
                    # enough not to delay sigmoid 0/1, early enough for the
                    # first select ops.
                    maskm = load(maskm_d, [128, POSCAP], F16, "maskm")
                    mks = load(mks_d, [128, NCH * POSCAP], F16, "mks")

                sk = spool.tile([128, CH], F16, tag="schunk", name="schunk")
                qk = spool.tile([128, CH], F16, tag="qchunk", name="qchunk")
                for h in range(2):
                    sl = slice(h * HCH, (h + 1) * HCH)
                    nc.scalar.activation(sk[:, sl], xk[:, sl], AF.Sigmoid)
                    nc.vector.tensor_tensor(qk[:, sl], sk[:, sl], sk[:, sl],
                                            ALU.mult)
                sgs.append(sk)
                qts.append(qk)

                gk = ppool.tile([128, POSCAP], F16, tag=f"g{k}", name=f"g{k}")
                nc.gpsimd.indirect_copy(gk[:], sk[:], idxm[:], True)
                gkt = pspool.tile([128, POSCAP], F16, tag=f"gt{k}",
                                  name=f"gt{k}")
                for t in range(T):
                    sl = slice(t * 128, (t + 1) * 128)
                    nc.tensor.transpose(gkt[:, sl], gk[:, sl], id16[:])
                gts.append(gkt)

                # chunk-(k-1) select/max steps (gathers trail sigmoids, so
                # work one chunk behind to avoid DVE stalls)
                if k >= 1:
                    kk = k - 1
                    sx = ppool.tile([128, POSCAP], F16, tag=f"sx{kk}",
                                    name=f"sx{kk}")
                    tt(sx[:], gts[kk][:], mks[:, kk * POSCAP:(kk + 1) * POSCAP],
                       ALU.mult)
                    if kk == 0:
                        ssel = sx
                        mx = ppool.tile([128, POSCAP], F16, tag="mx0",
                                        name="mx0")
                        nc.vector.tensor_copy(mx[:], gts[0][:])
                    else:
                        nsel = ppool.tile([128, POSCAP], F16, tag=f"ssel{kk}",
                                          name=f"ssel{kk}")
                        tt(nsel[:], ssel[:], sx[:], ALU.add)
                        ssel = nsel
                        nmx = ppool.tile([128, POSCAP], F16, tag=f"mx{kk}",
                                         name=f"mx{kk}")
                        tt(nmx[:], mx[:], gts[kk][:], ALU.max)
                        mx = nmx

                if k == 2:
                    # ---- bbox softmax / decode / IoU / GIoU (small tiles,
                    # inputs all ready; fills the DVE sigmoid window) ----
                    ome = vtile([128, T * 32], "ome")
                    nc.vector.tensor_scalar(ome[:], esg[:], -1.0, 1.0,
                                            ALU.mult, ALU.add)
                    re = vtile([128, T * 32], "re")
                    nc.vector.reciprocal(re[:], ome[:])
                    e = vtile([128, T * 32], "e")
                    tt(e[:], esg[:], re[:], ALU.mult)
                    S = vtile([128, T * 4], "S")
                    nc.vector.tensor_reduce(
                        S[:].rearrange("p (t k) -> p t k", t=T, k=4),
                        e[:].rearrange("p (t k j) -> p t k j", t=T, k=4, j=R1),
                        axis=AX.X, op=ALU.add)
                    we = vtile([128, T * 32], "we")
                    tt(we[:], e[:], jf[:], ALU.mult)
                    wS = vtile([128, T * 4], "wS")
                    nc.vector.tensor_reduce(
                        wS[:].rearrange("p (t k) -> p t k", t=T, k=4),
                        we[:].rearrange("p (t k j) -> p t k j", t=T, k=4,
                                        j=R1),
                        axis=AX.X, op=ALU.add)
                    rS = vtile([128, T * 4], "rS")
                    nc.vector.reciprocal(rS[:], S[:])
                    crn = vtile([128, T * 4], "crn")
                    tt(crn[:], wS[:], rS[:], ALU.mult)

                    rstr = vtile([128, 1], "rstr")
                    nc.vector.reciprocal(rstr[:], strd[:])
                    rsh = vtile([128, 1], "rsh")
                    nc.vector.tensor_scalar_mul(rsh[:], rstr[:], 0.5)
                    anc3 = anc[:].rearrange("p (t c) -> p t c", t=T, c=4)
                    ctr2 = vtile([128, T * 2], "ctr2")
                    ctr2v = ctr2[:].rearrange("p (t c) -> p t c", t=T, c=2)
                    tt(ctr2v, anc3[:, :, 0:2], anc3[:, :, 2:4], ALU.add)
                    ctr = vtile([128, T * 2], "ctr")
                    tt(ctr[:], ctr2[:], rsh[:].broadcast_to((128, T * 2)),
                       ALU.mult)
                    targ = vtile([128, T * 4], "targ")
                    tt(targ[:], tgt[:], rstr[:].broadcast_to((128, T * 4)),
                       ALU.mult)

                    ctrv = ctr[:].rearrange("p (t c) -> p t c", t=T, c=2)
                    crnv = crn[:].rearrange("p (t c) -> p t c", t=T, c=4)
                    targv = targ[:].rearrange("p (t c) -> p t c", t=T, c=4)

                    dec = vtile([128, T * 4], "dec")
                    decv = dec[:].rearrange("p (t c) -> p t c", t=T, c=4)
                    tt(decv[:, :, 0:2], ctrv, crnv[:, :, 0:2], ALU.subtract)
                    tt(decv[:, :, 2:4], ctrv, crnv[:, :, 2:4], ALU.add)

                    lt = vtile([128, T * 2], "lt")
                    tt(lt[:].rearrange("p (t c) -> p t c", t=T, c=2),
                       decv[:, :, 0:2], targv[:, :, 0:2], ALU.max)
                    rb = vtile([128, T * 2], "rb")
                    tt(rb[:].rearrange("p (t c) -> p t c", t=T, c=2),
                       decv[:, :, 2:4], targv[:, :, 2:4], ALU.min)
                    whr = vtile([128, T * 2], "whr")
                    tt(whr[:], rb[:], lt[:], ALU.subtract)
                    wh = vtile([128, T * 2], "wh")
                    nc.vector.tensor_scalar_max(wh[:], whr[:], 0.0)
                    whv = wh[:].rearrange("p (t c) -> p t c", t=T, c=2)
                    ov = vtile([128, T], "ov")
                    tt(ov[:].unsqueeze(2), whv[:, :, 0:1], whv[:, :, 1:2],
                       ALU.mult)

                    def area(tag, v):
                        w_ = vtile([128, T * 2], tag + "wh")
                        w_v = w_[:].rearrange("p (t c) -> p t c", t=T, c=2)
                        tt(w_v, v[:, :, 2:4], v[:, :, 0:2], ALU.subtract)
                        a_ = vtile([128, T], tag)
                        tt(a_[:].unsqueeze(2), w_v[:, :, 0:1],
                           w_v[:, :, 1:2], ALU.mult)
                        return a_

                    ap_ = area("ap", decv)
                    at_ = area("at", targv)
                    un = vtile([128, T], "un")
                    tt(un[:], ap_[:], at_[:], ALU.add)
                    tt(un[:], un[:], ov[:], ALU.subtract)
                    nc.vector.tensor_scalar_max(un[:], un[:], EPS)
                    run_ = vtile([128, T], "run")
                    nc.vector.reciprocal(run_[:], un[:])
                    iou = vtile([128, T], "iou")
                    tt(iou[:], ov[:], run_[:], ALU.mult)

                    elt = vtile([128, T * 2], "elt")
                    tt(elt[:].rearrange("p (t c) -> p t c", t=T, c=2),
                       decv[:, :, 0:2], targv[:, :, 0:2], ALU.min)
                    erb = vtile([128, T * 2], "erb")
                    tt(erb[:].rearrange("p (t c) -> p t c", t=T, c=2),
                       decv[:, :, 2:4], targv[:, :, 2:4], ALU.max)
                    ewr = vtile([128, T * 2], "ewr")
                    tt(ewr[:], erb[:], elt[:], ALU.subtract)
                    ew = vtile([128, T * 2], "ew")
                    nc.vector.tensor_scalar_max(ew[:], ewr[:], 0.0)
                    ewv = ew[:].rearrange("p (t c) -> p t c", t=T, c=2)
                    ea = vtile([128, T], "ea")
                    tt(ea[:].unsqueeze(2), ewv[:, :, 0:1], ewv[:, :, 1:2],
                       ALU.mult)
                    nc.vector.tensor_scalar_max(ea[:], ea[:], EPS)
                    rea = vtile([128, T], "rea")
                    nc.vector.reciprocal(rea[:], ea[:])
                    gd = vtile([128, T], "gd")
                    tt(gd[:], ea[:], un[:], ALU.subtract)
                    tt(gd[:], gd[:], rea[:], ALU.mult)
                    giou = vtile([128, T], "giou")
                    tt(giou[:], iou[:], gd[:], ALU.subtract)
                    og = vtile([128, T], "og")
                    nc.vector.tensor_scalar(og[:], giou[:], -1.0, 1.0,
                                            ALU.mult, ALU.add)

                if k == 3:
                    # ---- DFL targets (lse-independent part) ----
                    dist = vtile([128, T * 4], "dist")
                    distv = dist[:].rearrange("p (t c) -> p t c", t=T, c=4)
                    tt(distv[:, :, 0:2], ctrv, targv[:, :, 0:2], ALU.subtract)
                    tt(distv[:, :, 2:4], targv[:, :, 2:4], ctrv, ALU.subtract)
                    nc.vector.tensor_scalar_max(dist[:], dist[:], 0.0)
                    nc.vector.tensor_scalar_min(dist[:], dist[:], REG_TOP)
                    y = vtile([128, T * 32], "y")
                    tt(y[:].rearrange("p (t k j) -> p t k j", t=T, k=4, j=R1),
                       jf[:].rearrange("p (t k j) -> p t k j", t=T, k=4,
                                       j=R1),
                       dist[:].rearrange("p (t k) -> p t k", t=T, k=4)
                              .unsqueeze(3).broadcast_to((128, T, 4, R1)),
                       ALU.subtract)
                    yn = vtile([128, T * 32], "yn")
                    nc.vector.tensor_scalar_mul(yn[:], y[:], -1.0)
                    ya = vtile([128, T * 32], "ya")
                    tt(ya[:], y[:], yn[:], ALU.max)
                    tent = vtile([128, T * 32], "tent")
                    nc.vector.tensor_scalar(tent[:], ya[:], -1.0, 1.0,
                                            ALU.mult, ALU.add)
                    nc.vector.tensor_scalar_max(tent[:], tent[:], 0.0)
                    xt = vtile([128, T * 32], "xt")
                    tt(xt[:], bbc[:], tent[:], ALU.mult)
                    xts = vtile([128, T * 4], "xts")
                    nc.vector.tensor_reduce(
                        xts[:].rearrange("p (t k) -> p t k", t=T, k=4),
                        xt[:].rearrange("p (t k j) -> p t k j", t=T, k=4,
                                        j=R1),
                        axis=AX.X, op=ALU.add)

            # ---------------- chunk-4 select/max + wt + sxa -------------
            sx = ppool.tile([128, POSCAP], F16, tag="sx4", name="sx4")
            tt(sx[:], gts[4][:], mks[:, 4 * POSCAP:5 * POSCAP], ALU.mult)
            nsel = ppool.tile([128, POSCAP], F16, tag="ssel4", name="ssel4")
            tt(nsel[:], ssel[:], sx[:], ALU.add)
            ssel = nsel
            nmx = ppool.tile([128, POSCAP], F16, tag="mx4", name="mx4")
            tt(nmx[:], mx[:], gts[4][:], ALU.max)
            mx = nmx

            mm = ppool.tile([128, POSCAP], F16, tag="mm", name="mm")
            nc.vector.scalar_tensor_tensor(mm[:], mx[:], 2.0, maskm[:],
                                           ALU.add, ALU.mult)
            wtm = ppool.tile([128, T], F32, tag="wtm", name="wtm")
            nc.vector.tensor_reduce(
                wtm[:], mm[:].rearrange("p (t l) -> p t l", t=T, l=128),
                axis=AX.X, op=ALU.max)
            wt = ppool.tile([128, T], F32, tag="wt", name="wt")
            nc.vector.tensor_scalar_add(wt[:], wtm[:], -2.0)
            wtv = ppool.tile([128, T], F32, tag="wtv", name="wtv")
            tt(wtv[:], wt[:], wv[:], ALU.mult)
            lbs = vtile([128, T], "lbs")
            tt(lbs[:], og[:], wtv[:], ALU.mult)

            sxa = ppool.tile([128, T], F32, tag="sxa", name="sxa")
            nc.vector.tensor_reduce(
                sxa[:], ssel[:].rearrange("p (t l) -> p t l", t=T, l=128),
                axis=AX.X, op=ALU.add)
            sxl = vtile([128, T], "sxl")
            nc.vector.tensor_scalar_max(sxl[:], sxa[:], 1e-7)
            u2 = vtile([128, T], "u2")
            nc.vector.tensor_scalar(u2[:], sxl[:], -1.0, 1.0, ALU.mult,
                                    ALU.add)
            nc.vector.tensor_scalar_max(u2[:], u2[:], 1e-7)

            # ---------------- Act: softplus phase -----------------------
            sps = []
            for k in range(NCH):
                for h in range(2):
                    sl = slice(h * HCH, (h + 1) * HCH)
                    pk = dpool.tile([128, HCH], F16, tag="spchunk",
                                    name="spchunk", bufs=3)
                    nc.scalar.activation(pk[:], sgs[k][:, sl], AF.Ln,
                                         scale=-1.0, bias=lnb[:])
                    sps.append((k, h, pk))
            # remaining natural_log ops ride the same table set
            lse = vtile([128, T * 4], "lse")
            nc.scalar.activation(lse[:], S[:], AF.Ln)
            lns = vtile([128, T], "lns")
            nc.scalar.activation(lns[:], sxl[:], AF.Ln)
            ln1m = vtile([128, T], "ln1m")
            nc.scalar.activation(ln1m[:], u2[:], AF.Ln)

            # ---------------- DVE: dense f-sum (fused mult+reduce) ------
            # f = sp*q on DVE; partition-collapsing column sums accumulate on
            # the (otherwise idle) PE into one [1,512] PSUM bank.
            fpsum = pspool.tile([1, 512], F32, tag="fpsum", name="fpsum")
            nmm = len(sps) * (HCH // 512)
            mi = 0
            for (k, h, pk) in sps:
                sl = slice(h * HCH, (h + 1) * HCH)
                fkh = dpool.tile([128, HCH], F16, tag="fchunk", name="fchunk")
                tt(fkh[:], pk[:], qts[k][:, sl], ALU.mult)
                for s in range(HCH // 512):
                    nc.tensor.matmul(
                        out=fpsum[:], lhsT=ones16c[:],
                        rhs=fkh[:, s * 512:(s + 1) * 512],
                        start=(mi == 0), stop=(mi == nmm - 1))
                    mi += 1
            fs1 = vtile([1, 1], "fs1")
            nc.vector.tensor_reduce(fs1[:], fpsum[:], axis=AX.X, op=ALU.add)
            fsn = vtile([1, 1], "fsn")
            nc.vector.tensor_scalar_mul(fsn[:], fs1[:], -1.0)

            # ---------------- tail: DFL + QFL positive terms ------------
            dfk = vtile([128, T * 4], "dfk")
            tt(dfk[:], lse[:], xts[:], ALU.subtract)
            dfr = vtile([128, T], "dfr")
            nc.vector.tensor_reduce(
                dfr[:], dfk[:].rearrange("p (t k) -> p t k", t=T, k=4),
                axis=AX.X, op=ALU.add)
            dfs = vtile([128, T], "dfs")
            tt(dfs[:], dfr[:], wtv[:], ALU.mult)

            xa = vtile([128, T], "xa")
            tt(xa[:], lns[:], ln1m[:], ALU.subtract)
            spxa = vtile([128, T], "spxa")
            nc.vector.tensor_scalar_mul(spxa[:], ln1m[:], -1.0)
            sxa2 = vtile([128, T], "sxa2")
            tt(sxa2[:], sxl[:], sxl[:], ALU.mult)
            fxa = vtile([128, T], "fxa")
            tt(fxa[:], sxa2[:], spxa[:], ALU.mult)
            xsc = vtile([128, T], "xsc")
            tt(xsc[:], xa[:], iou[:], ALU.mult)
            bce = vtile([128, T], "bce")
            tt(bce[:], spxa[:], xsc[:], ALU.subtract)
            sf = vtile([128, T], "sf")
            tt(sf[:], iou[:], sxl[:], ALU.subtract)
            sf2 = vtile([128, T], "sf2")
            tt(sf2[:], sf[:], sf[:], ALU.mult)
            pl = vtile([128, T], "pl")
            tt(pl[:], bce[:], sf2[:], ALU.mult)
            qc = vtile([128, T], "qc")
            tt(qc[:], pl[:], fxa[:], ALU.subtract)
            tt(qc[:], qc[:], lwv[:], ALU.mult)

            # ---------------- final partials ----------------
            def redcol(tag, src):
                o = vtile([128, 1], tag)
                nc.vector.tensor_reduce(o[:], src[:], axis=AX.X, op=ALU.add)
                return o

            qa = redcol("qa", qc)
            lba = redcol("lba", lbs)
            dfa = redcol("dfa", dfs)
            wta = redcol("wta", wtv)

            fin = vtile([128, 8], "fin")
            nc.vector.memset(fin[:], 0.0)
            nc.vector.tensor_copy(fin[:, 1:2], qa[:])
            nc.vector.tensor_copy(fin[:, 2:3], lba[:])
            nc.vector.tensor_copy(fin[:, 3:4], dfa[:])
            nc.vector.tensor_copy(fin[:, 4:5], wta[:])

            outp = pspool.tile([1, 8], F32, tag="outp", name="outp")
            nc.tensor.matmul(out=outp[:], lhsT=ones_col[:], rhs=fin[:],
                             start=True, stop=True)
            outs = vtile([1, 8], "outs")
            nc.vector.tensor_copy(outs[:], outp[:])
            nc.vector.tensor_copy(outs[:, 0:1], fsn[:])
            nc.sync.dma_start(out=out_d[:], in_=outs[:])

    return nc


_NC = None


def _get_nc():
    global _NC
    if _NC is None:
        _NC = build_nc()
    return _NC


def make_in_maps(anchors, cls_score, bbox_pred, label_weights, bbox_targets,
                 labels):
    """Host-side sharding + positive-row compaction (pure indexing)."""
    cls_score = np.ascontiguousarray(cls_score, np.float32)
    bbox_pred = np.ascontiguousarray(bbox_pred, np.float32)
    labels = np.asarray(labels, np.int32)
    label_weights = np.asarray(label_weights, np.float32)
    bbox_targets = np.asarray(bbox_targets, np.float32)
    anchors = np.asarray(anchors, np.float32)

    def fold(v):  # [POSCAP, k] -> [128, T*k] with slot i = p + 128*t
        k = v.shape[1] if v.ndim > 1 else 1
        return np.ascontiguousarray(
            v.reshape(T, 128, k).transpose(1, 0, 2).reshape(128, T * k))

    def wrap16(idx):  # uint16 wrapped index layout, replicated per 16-group
        w = idx.reshape(POSCAP // 16, 16).T.astype(np.uint16)  # [16, 64]
        return np.ascontiguousarray(np.tile(w, (8, 1)))

    jfv = np.ascontiguousarray(
        np.broadcast_to(np.tile(np.arange(R1, dtype=np.float32), T * 4),
                        (128, T * 4 * R1)))
    cgrid = np.arange(128)

    in_maps = []
    for r in range(NCORES):
        base = r * NPC
        lab = labels[base:base + NPC]
        pos = np.nonzero(lab < C)[0]
        npos = len(pos)
        assert npos <= POSCAP, f"positive count {npos} exceeds cap {POSCAP}"
        idx = np.zeros(POSCAP, np.int64)
        idx[:npos] = pos
        valid = np.zeros(POSCAP, np.float32)
        valid[:npos] = 1.0
        b_loc = idx // HW
        hw = idx % HW
        labp = np.where(valid > 0, lab[idx], 0).astype(np.int64)
        gidx = base + idx

        bbc = bbox_pred.reshape(B, 32, HW)[r * BPC + b_loc, :, hw]  # [P, 32]
        tgt = bbox_targets[gidx]                                    # [P, 4]
        anc = anchors[gidx]                                         # [P, 4]
        lwv = label_weights[gidx] * valid
        blocv = np.where(valid > 0, b_loc, -1).astype(np.int64)

        # post-transpose masks: slot i = p + 128*t lives at [p, t*128 + c],
        # c = source slab row. maskm picks the 16 rows of the slot's image;
        # mk_k picks row 16*b + lab//5 iff lab%5 == k.
        blocf = fold(blocv[:, None])            # [128, T]
        maskm = (cgrid[None, None, :] // 16 == blocf[:, :, None])
        maskm = np.ascontiguousarray(
            maskm.reshape(128, POSCAP).astype(np.float16))
        mks = []
        for k in range(NCH):
            qsel = np.where((valid > 0) & (labp % NCH == k),
                            16 * b_loc + labp // NCH, -1).astype(np.int64)
            qself = fold(qsel[:, None])         # [128, T]
            mk = (cgrid[None, None, :] == qself[:, :, None])
            mks.append(mk.reshape(128, POSCAP).astype(np.float16))
        mks = np.ascontiguousarray(np.concatenate(mks, axis=1))

        in_maps.append({
            "cls": cls_score[r * BPC:(r + 1) * BPC]
                .reshape(128, ROWF).astype(np.float16),
            "bbc": fold(bbc),
            "tgt": fold(tgt),
            "anc": fold(anc),
            "wv": fold(valid[:, None]),
            "lwv": fold(lwv[:, None]),
            "jfv": jfv,
            "maskm": maskm,
            "mks": mks,
            "idxm": wrap16(hw),
            "strd": np.zeros((128, 1), np.float32),  # patched by caller
        })
    return in_maps


def combine(results, num_total_samples):
    tot = np.zeros(8, np.float64)
    for r in results:
        tot += r["out"].reshape(8).astype(np.float64)
    qfl = (tot[0] + tot[1]) / float(num_total_samples)
    bbox = 2.0 * tot[2]
    dfl = tot[3] * 0.0625
    wsum = tot[4]
    return np.array([qfl, bbox, dfl, wsum], np.float32)


def kernel(anchors, cls_score, bbox_pred, label_weights, bbox_targets,
           labels, num_total_samples, stride):
    in_maps = make_in_maps(anchors, cls_score, bbox_pred, label_weights,
                           bbox_targets, labels)
    for m in in_maps:
        m["strd"] = np.full((128, 1), float(stride), np.float32)
    nc = _get_nc()
    res = run_bass_kernel_spmd(nc, in_maps, list(range(NCORES)))
    return combine(res.results, num_total_samples)


if __name__ == "__main__":
    pass


# revision 14
# speedup vs baseline: 6.4850x; 2.9497x over previous
"""Trainium2 Bass kernel for NanodetLoss (nn_NanodetLoss_89343909692049).

Strategy (v3)
-------------
Data-parallel over batch: core r handles images [8r, 8r+8), i.e. a
contiguous 32768-pixel slab of the flattened N = B*H*W axis.

The loss decomposes as
  qfl  = [ sum_{n,c} f(x_nc)  +  sum_{pos} lw*(pos_loss - f(x_at_lab)) ] / num_total
  bbox = 2    * sum_{pos} (1-giou)*wt
  dfl  = 1/16 * sum_{pos,k} dfl_k*wt
  wsum =        sum_{pos} wt
with f(x) = softplus(x)*sigmoid(x)^2 and wt = max_c sigmoid(x) at positives.

Dense pipeline per core (cls slab [128, 20480] fp16, streamed in
2048-wide halves):
  Act: s = Sigmoid(x) fp16            (one table set)
  DVE: q = s*s                        (runs inside the sigmoid window)
  Act: sp = -Ln((1+2^-23) - s) = softplus(x)   (one table switch, total)
  DVE: f = sp*q
  PE : ones^T @ f accumulated into one [1,512] PSUM bank (the row-sum),
       finished by one small DVE reduce.
All positive-anchor tensors (~2% of pixels) are host-compacted by pure
indexing, including the positive pixels' 80 class logits (xpos) and the
logit at the label (xat) — so the positive branch needs no on-device
gather at all: wt = sigmoid(max_c logit) via one [128,640] reduce, and
the QFL/GIoU/DFL terms run on tiny [128, 8*k] tiles. exp(x) for the
bbox softmax is sigma(x)/(1-sigma(x)), so only two activation table
sets load in total. Per-core output is a [1,8] vector of partial sums;
the host adds the 8 vectors and applies the scalar normalizations
(pure epilogue).
"""

import sys

for _p in ("/opt/trn_rl_repo",):
    if _p not in sys.path:
        sys.path.insert(0, _p)

import numpy as np

import concourse.bass as bass
import concourse.mybir as mybir
from concourse.tile import TileContext
from concourse.vector_clock import ScopedClock
from concourse.bass_utils import run_bass_kernel_spmd

F32 = mybir.dt.float32
F16 = mybir.dt.float16
I32 = mybir.dt.int32
AF = mybir.ActivationFunctionType
ALU = mybir.AluOpType
AX = mybir.AxisListType

# Problem geometry (fixed by the task spec).
B, C, R1 = 64, 80, 8
H = W = 64
HW = H * W                 # 4096
NCORES = 8
BPC = B // NCORES          # 8 batches per core
NPC = BPC * HW             # 32768 pixels per core
ROWF = BPC * C * HW // 128  # 20480 elements per SBUF row of the flat cls slab
CH = HW                    # channel-slice size (one channel per row): 4096
NCH = ROWF // CH           # 5
HCH = CH // 2              # 2048-wide streaming halves
POSCAP = 1024              # padded positive-slot capacity per core
T = POSCAP // 128          # 8 slot columns
REG_TOP = R1 - 1 - 0.1     # 6.9 bbox2distance clamp
EPS = 1e-6
LNB = 1.0 + 2.0 ** -23     # softplus ln bias; guards ln(0) at sigma==1
NEGX = -40.0               # pad logit for invalid slots: sigma -> 0


class _SplitDrainTileContext(TileContext):
    """This container's walrus build rejects instructions carrying more than
    one sync-wait. Tile's wait assignment freely emits multi-waits, so after
    scheduling we hoist all but one wait of each instruction onto NOPs
    inserted right before it on the same engine (waiting earlier on the same
    engine is equivalent: every hoisted wait was already required there)."""

    def _drain_and_barrier(self, tick_clock, wait_clock):
        drain_inst = self.nc.sync.drain()
        wait_clock.add_sem_waits(
            drain_inst.ins, ScopedClock({None: tick_clock.global_clock})
        )
        waits = list(drain_inst.ins.sync_info.on_wait)
        if len(waits) > 1:
            drain_inst.ins.sync_info.on_wait = waits[:1]
            for w in waits[1:]:
                d2 = self.nc.sync.drain()
                d2.ins.sync_info = mybir.SyncInfo(on_wait=[w], on_update=[])
        self.nc.all_engine_barrier()
        assert self.sems is not None
        popped = self.nc._tile_sem_poison_stack.pop()
        assert popped is self._sem_poison
        self.nc.clear_and_free_semaphores(list(self.sems.allocated().values()))
        self.nc.all_engine_barrier()

    def schedule_and_allocate(self):
        ret = super().schedule_and_allocate()
        nc = self.nc
        for bb_name, bbw in list(nc.bb_map.items()):
            bb = bbw.bb
            insts = bb.instructions
            out = []
            changed = False
            for inst in insts:
                si = inst.sync_info
                if si is not None and si.on_wait and len(si.on_wait) > 1:
                    waits = list(si.on_wait)
                    for w in waits[:-1]:
                        nop = mybir.InstNoOp(
                            name=f"waitnop-{nc.next_id()}",
                            engine=inst.engine,
                            bass_nofuse=True,
                            sync_info=mybir.SyncInfo(on_wait=[w], on_update=[]),
                        )
                        nc.register_instruction(nop)
                        out.append(nop)
                    inst.sync_info = mybir.SyncInfo(
                        on_wait=[waits[-1]], on_update=list(si.on_update))
                    changed = True
                out.append(inst)
            if changed:
                bb.instructions = out
        return ret


def build_nc():
    nc = bass.Bass("TRN2", target_bir_lowering=False, debug=False,
                   num_devices=NCORES)

    cls_d = nc.dram_tensor("cls", [128, ROWF], F16, kind="ExternalInput")
    bbc_d = nc.dram_tensor("bbc", [128, T * 4 * R1], F32, kind="ExternalInput")
    xpos_d = nc.dram_tensor("xpos", [128, T * C], F32, kind="ExternalInput")
    xat_d = nc.dram_tensor("xat", [128, T], F32, kind="ExternalInput")
    tgt_d = nc.dram_tensor("tgt", [128, T * 4], F32, kind="ExternalInput")
    anc_d = nc.dram_tensor("anc", [128, T * 4], F32, kind="ExternalInput")
    wv_d = nc.dram_tensor("wv", [128, T], F32, kind="ExternalInput")
    lwv_d = nc.dram_tensor("lwv", [128, T], F32, kind="ExternalInput")
    jf_d = nc.dram_tensor("jfv", [128, T * 4 * R1], F32, kind="ExternalInput")
    strd_d = nc.dram_tensor("strd", [128, 1], F32, kind="ExternalInput")
    out_d = nc.dram_tensor("out", [1, 8], F32, kind="ExternalOutput")

    with _SplitDrainTileContext(nc) as tc:
        with (
            tc.tile_pool(name="const", bufs=1) as cpool,
            tc.tile_pool(name="xc", bufs=3) as xpool,
            tc.tile_pool(name="sg", bufs=5) as spool,
            tc.tile_pool(name="dense", bufs=2) as dpool,
            tc.tile_pool(name="pos", bufs=1) as ppool,
            tc.tile_pool(name="ps", bufs=1, space="PSUM") as pspool,
        ):
            def vtile(shape, tag):
                return ppool.tile(shape, F32, tag=tag, name=tag)

            def tt(out, a, b, op):
                nc.vector.tensor_tensor(out, a, b, op)

            # ---------------- constants (gpsimd) ----------------
            ones_col = cpool.tile([128, 1], F32, tag="ones_col", name="ones_col")
            nc.gpsimd.memset(ones_col[:], 1.0)
            lnb = cpool.tile([128, 1], F32, tag="lnb", name="lnb")
            nc.gpsimd.memset(lnb[:], LNB)
            ones16c = cpool.tile([128, 1], F16, tag="ones16c", name="ones16c")
            nc.gpsimd.memset(ones16c[:], 1.0)

            # ---------------- small input loads ----------------
            def load(dram, shape, dtype, tag):
                t = cpool.tile(shape, dtype, tag=tag, name=tag)
                nc.sync.dma_start(out=t[:], in_=dram[:])
                return t

            bbc = load(bbc_d, [128, T * 4 * R1], F32, "bbc")
            xpos = load(xpos_d, [128, T * C], F32, "xpos")
            xat = load(xat_d, [128, T], F32, "xat")
            tgt = load(tgt_d, [128, T * 4], F32, "tgt")
            anc = load(anc_d, [128, T * 4], F32, "anc")
            wv = load(wv_d, [128, T], F32, "wv")
            lwv = load(lwv_d, [128, T], F32, "lwv")
            jf = load(jf_d, [128, T * 4 * R1], F32, "jf")
            strd = load(strd_d, [128, 1], F32, "strd")

            # ---------------- positive sigma inputs (DVE+Act, tiny) -----
            wtl = vtile([128, T], "wtl")
            nc.vector.tensor_reduce(
                wtl[:], xpos[:].rearrange("p (t c) -> p t c", t=T, c=C),
                axis=AX.X, op=ALU.max)

            # ---------------- Act phase 0: sigmoid set ------------------
            dummy = cpool.tile([128, 1], F32, tag="dummy", name="dummy")
            nc.scalar.activation(dummy[:], ones_col[:], AF.Sigmoid)
            # exp(x) for the bbox softmax via sigma/(1-sigma).
            esg = ppool.tile([128, T * 32], F32, tag="esg", name="esg")
            nc.scalar.activation(esg[:], bbc[:], AF.Sigmoid)
            wt = vtile([128, T], "wt")
            nc.scalar.activation(wt[:], wtl[:], AF.Sigmoid)
            sat = vtile([128, T], "sat")
            nc.scalar.activation(sat[:], xat[:], AF.Sigmoid)

            # ---------------- dense stream: sigmoid + q = s^2 -----------
            sgs, qts = [], []
            for k in range(NCH):
                xk = xpool.tile([128, CH], F16, tag="xchunk", name="xchunk")
                for h in range(2):
                    sl = slice(h * HCH, (h + 1) * HCH)
                    nc.sync.dma_start(out=xk[:, sl],
                                      in_=cls_d[:, k * CH + h * HCH:
                                                k * CH + (h + 1) * HCH])
                sk = spool.tile([128, CH], F16, tag="schunk", name="schunk")
                qk = spool.tile([128, CH], F16, tag="qchunk", name="qchunk")
                for h in range(2):
                    sl = slice(h * HCH, (h + 1) * HCH)
                    nc.scalar.activation(sk[:, sl], xk[:, sl], AF.Sigmoid)
                    nc.vector.tensor_tensor(qk[:, sl], sk[:, sl], sk[:, sl],
                                            ALU.mult)
                sgs.append(sk)
                qts.append(qk)

                if k == 1:
                    # ---- bbox softmax / decode / IoU / GIoU (small tiles,
                    # inputs all ready; fills the DVE sigmoid window) ----
                    wtv = vtile([128, T], "wtv")
                    tt(wtv[:], wt[:], wv[:], ALU.mult)

                    ome = vtile([128, T * 32], "ome")
                    nc.vector.tensor_scalar(ome[:], esg[:], -1.0, 1.0,
                                            ALU.mult, ALU.add)
                    re = vtile([128, T * 32], "re")
                    nc.vector.reciprocal(re[:], ome[:])
                    e = vtile([128, T * 32], "e")
                    tt(e[:], esg[:], re[:], ALU.mult)
                    S = vtile([128, T * 4], "S")
                    nc.vector.tensor_reduce(
                        S[:].rearrange("p (t k) -> p t k", t=T, k=4),
                        e[:].rearrange("p (t k j) -> p t k j", t=T, k=4, j=R1),
                        axis=AX.X, op=ALU.add)
                    we = vtile([128, T * 32], "we")
                    tt(we[:], e[:], jf[:], ALU.mult)
                    wS = vtile([128, T * 4], "wS")
                    nc.vector.tensor_reduce(
                        wS[:].rearrange("p (t k) -> p t k", t=T, k=4),
                        we[:].rearrange("p (t k j) -> p t k j", t=T, k=4,
                                        j=R1),
                        axis=AX.X, op=ALU.add)
                    rS = vtile([128, T * 4], "rS")
                    nc.vector.reciprocal(rS[:], S[:])
                    crn = vtile([128, T * 4], "crn")
                    tt(crn[:], wS[:], rS[:], ALU.mult)

                    rstr = vtile([128, 1], "rstr")
                    nc.vector.reciprocal(rstr[:], strd[:])
                    rsh = vtile([128, 1], "rsh")
                    nc.vector.tensor_scalar_mul(rsh[:], rstr[:], 0.5)
                    anc3 = anc[:].rearrange("p (t c) -> p t c", t=T, c=4)
                    ctr2 = vtile([128, T * 2], "ctr2")
                    ctr2v = ctr2[:].rearrange("p (t c) -> p t c", t=T, c=2)
                    tt(ctr2v, anc3[:, :, 0:2], anc3[:, :, 2:4], ALU.add)
                    ctr = vtile([128, T * 2], "ctr")
                    tt(ctr[:], ctr2[:], rsh[:].broadcast_to((128, T * 2)),
                       ALU.mult)
                    targ = vtile([128, T * 4], "targ")
                    tt(targ[:], tgt[:], rstr[:].broadcast_to((128, T * 4)),
                       ALU.mult)

                    ctrv = ctr[:].rearrange("p (t c) -> p t c", t=T, c=2)
                    crnv = crn[:].rearrange("p (t c) -> p t c", t=T, c=4)
                    targv = targ[:].rearrange("p (t c) -> p t c", t=T, c=4)

                    dec = vtile([128, T * 4], "dec")
                    decv = dec[:].rearrange("p (t c) -> p t c", t=T, c=4)
                    tt(decv[:, :, 0:2], ctrv, crnv[:, :, 0:2], ALU.subtract)
                    tt(decv[:, :, 2:4], ctrv, crnv[:, :, 2:4], ALU.add)

                    lt = vtile([128, T * 2], "lt")
                    tt(lt[:].rearrange("p (t c) -> p t c", t=T, c=2),
                       decv[:, :, 0:2], targv[:, :, 0:2], ALU.max)
                    rb = vtile([128, T * 2], "rb")
                    tt(rb[:].rearrange("p (t c) -> p t c", t=T, c=2),
                       decv[:, :, 2:4], targv[:, :, 2:4], ALU.min)
                    whr = vtile([128, T * 2], "whr")
                    tt(whr[:], rb[:], lt[:], ALU.subtract)
                    wh = vtile([128, T * 2], "wh")
                    nc.vector.tensor_scalar_max(wh[:], whr[:], 0.0)
                    whv = wh[:].rearrange("p (t c) -> p t c", t=T, c=2)
                    ov = vtile([128, T], "ov")
                    tt(ov[:].unsqueeze(2), whv[:, :, 0:1], whv[:, :, 1:2],
                       ALU.mult)

                    def area(tag, v):
                        w_ = vtile([128, T * 2], tag + "wh")
                        w_v = w_[:].rearrange("p (t c) -> p t c", t=T, c=2)
                        tt(w_v, v[:, :, 2:4], v[:, :, 0:2], ALU.subtract)
                        a_ = vtile([128, T], tag)
                        tt(a_[:].unsqueeze(2), w_v[:, :, 0:1],
                           w_v[:, :, 1:2], ALU.mult)
                        return a_

                    ap_ = area("ap", decv)
                    at_ = area("at", targv)
                    un = vtile([128, T], "un")
                    tt(un[:], ap_[:], at_[:], ALU.add)
                    tt(un[:], un[:], ov[:], ALU.subtract)
                    nc.vector.tensor_scalar_max(un[:], un[:], EPS)
                    run_ = vtile([128, T], "run")
                    nc.vector.reciprocal(run_[:], un[:])
                    iou = vtile([128, T], "iou")
                    tt(iou[:], ov[:], run_[:], ALU.mult)

                    elt = vtile([128, T * 2], "elt")
                    tt(elt[:].rearrange("p (t c) -> p t c", t=T, c=2),
                       decv[:, :, 0:2], targv[:, :, 0:2], ALU.min)
                    erb = vtile([128, T * 2], "erb")
                    tt(erb[:].rearrange("p (t c) -> p t c", t=T, c=2),
                       decv[:, :, 2:4], targv[:, :, 2:4], ALU.max)
                    ewr = vtile([128, T * 2], "ewr")
                    tt(ewr[:], erb[:], elt[:], ALU.subtract)
                    ew = vtile([128, T * 2], "ew")
                    nc.vector.tensor_scalar_max(ew[:], ewr[:], 0.0)
                    ewv = ew[:].rearrange("p (t c) -> p t c", t=T, c=2)
                    ea = vtile([128, T], "ea")
                    tt(ea[:].unsqueeze(2), ewv[:, :, 0:1], ewv[:, :, 1:2],
                       ALU.mult)
                    nc.vector.tensor_scalar_max(ea[:], ea[:], EPS)
                    rea = vtile([128, T], "rea")
                    nc.vector.reciprocal(rea[:], ea[:])
                    gd = vtile([128, T], "gd")
                    tt(gd[:], ea[:], un[:], ALU.subtract)
                    tt(gd[:], gd[:], rea[:], ALU.mult)
                    giou = vtile([128, T], "giou")
                    tt(giou[:], iou[:], gd[:], ALU.subtract)
                    og = vtile([128, T], "og")
                    nc.vector.tensor_scalar(og[:], giou[:], -1.0, 1.0,
                                            ALU.mult, ALU.add)
                    lbs = vtile([128, T], "lbs")
                    tt(lbs[:], og[:], wtv[:], ALU.mult)

                if k == 2:
                    # ---- DFL targets (lse-independent part) ----
                    dist = vtile([128, T * 4], "dist")
                    distv = dist[:].rearrange("p (t c) -> p t c", t=T, c=4)
                    tt(distv[:, :, 0:2], ctrv, targv[:, :, 0:2], ALU.subtract)
                    tt(distv[:, :, 2:4], targv[:, :, 2:4], ctrv, ALU.subtract)
                    nc.vector.tensor_scalar_max(dist[:], dist[:], 0.0)
                    nc.vector.tensor_scalar_min(dist[:], dist[:], REG_TOP)
                    y = vtile([128, T * 32], "y")
                    tt(y[:].rearrange("p (t k j) -> p t k j", t=T, k=4, j=R1),
                       jf[:].rearrange("p (t k j) -> p t k j", t=T, k=4,
                                       j=R1),
                       dist[:].rearrange("p (t k) -> p t k", t=T, k=4)
                              .unsqueeze(3).broadcast_to((128, T, 4, R1)),
                       ALU.subtract)
                    yn = vtile([128, T * 32], "yn")
                    nc.vector.tensor_scalar_mul(yn[:], y[:], -1.0)
                    ya = vtile([128, T * 32], "ya")
                    tt(ya[:], y[:], yn[:], ALU.max)
                    tent = vtile([128, T * 32], "tent")
                    nc.vector.tensor_scalar(tent[:], ya[:], -1.0, 1.0,
                                            ALU.mult, ALU.add)
                    nc.vector.tensor_scalar_max(tent[:], tent[:], 0.0)
                    xt = vtile([128, T * 32], "xt")
                    tt(xt[:], bbc[:], tent[:], ALU.mult)
                    xts = vtile([128, T * 4], "xts")
                    nc.vector.tensor_reduce(
                        xts[:].rearrange("p (t k) -> p t k", t=T, k=4),
                        xt[:].rearrange("p (t k j) -> p t k j", t=T, k=4,
                                        j=R1),
                        axis=AX.X, op=ALU.add)

                if k == 3:
                    # ---- QFL positive pieces not needing Ln ----
                    sxl = vtile([128, T], "sxl")
                    nc.vector.tensor_scalar_max(sxl[:], sat[:], 1e-7)
                    u2 = vtile([128, T], "u2")
                    nc.vector.tensor_scalar(u2[:], sxl[:], -1.0, 1.0,
                                            ALU.mult, ALU.add)
                    nc.vector.tensor_scalar_max(u2[:], u2[:], 1e-7)
                    xsc = vtile([128, T], "xsc")
                    tt(xsc[:], xat[:], iou[:], ALU.mult)
                    sf = vtile([128, T], "sf")
                    tt(sf[:], iou[:], sxl[:], ALU.subtract)
                    sf2 = vtile([128, T], "sf2")
                    tt(sf2[:], sf[:], sf[:], ALU.mult)
                    sxa2 = vtile([128, T], "sxa2")
                    tt(sxa2[:], sxl[:], sxl[:], ALU.mult)

            # ---------------- Act: softplus phase -----------------------
            sps = []
            for k in range(NCH):
                for h in range(2):
                    sl = slice(h * HCH, (h + 1) * HCH)
                    pk = dpool.tile([128, HCH], F16, tag="spchunk",
                                    name="spchunk", bufs=3)
                    nc.scalar.activation(pk[:], sgs[k][:, sl], AF.Ln,
                                         scale=-1.0, bias=lnb[:])
                    sps.append((k, h, pk))
            # remaining natural_log ops ride the same table set
            lse = vtile([128, T * 4], "lse")
            nc.scalar.activation(lse[:], S[:], AF.Ln)
            ln1m = vtile([128, T], "ln1m")
            nc.scalar.activation(ln1m[:], u2[:], AF.Ln)

            # ---------------- DVE+PE: dense f-sum -----------------------
            fpsum = pspool.tile([1, 512], F32, tag="fpsum", name="fpsum")
            nmm = len(sps) * (HCH // 512)
            mi = 0
            for (k, h, pk) in sps:
                sl = slice(h * HCH, (h + 1) * HCH)
                fkh = dpool.tile([128, HCH], F16, tag="fchunk", name="fchunk")
                tt(fkh[:], pk[:], qts[k][:, sl], ALU.mult)
                for s in range(HCH // 512):
                    nc.tensor.matmul(
                        out=fpsum[:], lhsT=ones16c[:],
                        rhs=fkh[:, s * 512:(s + 1) * 512],
                        start=(mi == 0), stop=(mi == nmm - 1))
                    mi += 1
            fs1 = vtile([1, 1], "fs1")
            nc.vector.tensor_reduce(fs1[:], fpsum[:], axis=AX.X, op=ALU.add)
            fsn = vtile([1, 1], "fsn")
            nc.vector.tensor_scalar_mul(fsn[:], fs1[:], -1.0)

            # ---------------- tail: DFL + QFL positive terms ------------
            dfk = vtile([128, T * 4], "dfk")
            tt(dfk[:], lse[:], xts[:], ALU.subtract)
            dfr = vtile([128, T], "dfr")
            nc.vector.tensor_reduce(
                dfr[:], dfk[:].rearrange("p (t k) -> p t k", t=T, k=4),
                axis=AX.X, op=ALU.add)
            dfs = vtile([128, T], "dfs")
            tt(dfs[:], dfr[:], wtv[:], ALU.mult)

            spxa = vtile([128, T], "spxa")
            nc.vector.tensor_scalar_mul(spxa[:], ln1m[:], -1.0)
            fxa = vtile([128, T], "fxa")
            tt(fxa[:], sxa2[:], spxa[:], ALU.mult)
            bce = vtile([128, T], "bce")
            tt(bce[:], spxa[:], xsc[:], ALU.subtract)
            pl = vtile([128, T], "pl")
            tt(pl[:], bce[:], sf2[:], ALU.mult)
            qc = vtile([128, T], "qc")
            tt(qc[:], pl[:], fxa[:], ALU.subtract)
            tt(qc[:], qc[:], lwv[:], ALU.mult)

            # ---------------- final partials ----------------
            def redcol(tag, src):
                o = vtile([128, 1], tag)
                nc.vector.tensor_reduce(o[:], src[:], axis=AX.X, op=ALU.add)
                return o

            qa = redcol("qa", qc)
            lba = redcol("lba", lbs)
            dfa = redcol("dfa", dfs)
            wta = redcol("wta", wtv)

            fin = vtile([128, 8], "fin")
            nc.vector.memset(fin[:], 0.0)
            nc.vector.tensor_copy(fin[:, 1:2], qa[:])
            nc.vector.tensor_copy(fin[:, 2:3], lba[:])
            nc.vector.tensor_copy(fin[:, 3:4], dfa[:])
            nc.vector.tensor_copy(fin[:, 4:5], wta[:])

            outp = pspool.tile([1, 8], F32, tag="outp", name="outp")
            nc.tensor.matmul(out=outp[:], lhsT=ones_col[:], rhs=fin[:],
                             start=True, stop=True)
            outs = vtile([1, 8], "outs")
            nc.vector.tensor_copy(outs[:], outp[:])
            nc.vector.tensor_copy(outs[:, 0:1], fsn[:])
            nc.sync.dma_start(out=out_d[:], in_=outs[:])

    return nc


_NC = None


def _get_nc():
    global _NC
    if _NC is None:
        _NC = build_nc()
    return _NC


def make_in_maps(anchors, cls_score, bbox_pred, label_weights, bbox_targets,
                 labels):
    """Host-side sharding + positive-row compaction (pure indexing)."""
    cls_score = np.ascontiguousarray(cls_score, np.float32)
    bbox_pred = np.ascontiguousarray(bbox_pred, np.float32)
    labels = np.asarray(labels, np.int32)
    label_weights = np.asarray(label_weights, np.float32)
    bbox_targets = np.asarray(bbox_targets, np.float32)
    anchors = np.asarray(anchors, np.float32)

    def fold(v):  # [POSCAP, k] -> [128, T*k] with slot i = p + 128*t
        k = v.shape[1] if v.ndim > 1 else 1
        return np.ascontiguousarray(
            v.reshape(T, 128, k).transpose(1, 0, 2).reshape(128, T * k))

    jfv = np.ascontiguousarray(
        np.broadcast_to(np.tile(np.arange(R1, dtype=np.float32), T * 4),
                        (128, T * 4 * R1)))

    in_maps = []
    for r in range(NCORES):
        base = r * NPC
        lab = labels[base:base + NPC]
        pos = np.nonzero(lab < C)[0]
        npos = len(pos)
        assert npos <= POSCAP, f"positive count {npos} exceeds cap {POSCAP}"
        idx = np.zeros(POSCAP, np.int64)
        idx[:npos] = pos
        valid = np.zeros(POSCAP, np.float32)
        valid[:npos] = 1.0
        b_loc = idx // HW
        hw = idx % HW
        labp = np.where(valid > 0, lab[idx], 0).astype(np.int64)
        gidx = base + idx

        bbc = bbox_pred.reshape(B, 32, HW)[r * BPC + b_loc, :, hw]  # [P, 32]
        csr = cls_score.reshape(B, C, HW)
        xpos = csr[r * BPC + b_loc, :, hw]                          # [P, 80]
        xpos[valid == 0] = NEGX
        xatv = csr[r * BPC + b_loc, labp, hw]                       # [P]
        xatv[valid == 0] = NEGX
        tgt = bbox_targets[gidx]                                    # [P, 4]
        anc = anchors[gidx]                                         # [P, 4]
        lwv = label_weights[gidx] * valid

        in_maps.append({
            "cls": cls_score[r * BPC:(r + 1) * BPC]
                .reshape(128, ROWF).astype(np.float16),
            "bbc": fold(bbc),
            "xpos": fold(xpos),
            "xat": fold(xatv[:, None]),
            "tgt": fold(tgt),
            "anc": fold(anc),
            "wv": fold(valid[:, None]),
            "lwv": fold(lwv[:, None]),
            "jfv": jfv,
            "strd": np.zeros((128, 1), np.float32),  # patched by caller
        })
    return in_maps


def combine(results, num_total_samples):
    tot = np.zeros(8, np.float64)
    for r in results:
        tot += r["out"].reshape(8).astype(np.float64)
    qfl = (tot[0] + tot[1]) / float(num_total_samples)
    bbox = 2.0 * tot[2]
    dfl = tot[3] * 0.0625
    wsum = tot[4]
    return np.array([qfl, bbox, dfl, wsum], np.float32)


def kernel(anchors, cls_score, bbox_pred, label_weights, bbox_targets,
           labels, num_total_samples, stride):
    in_maps = make_in_maps(anchors, cls_score, bbox_pred, label_weights,
                           bbox_targets, labels)
    for m in in_maps:
        m["strd"] = np.full((128, 1), float(stride), np.float32)
    nc = _get_nc()
    res = run_bass_kernel_spmd(nc, in_maps, list(range(NCORES)))
    return combine(res.results, num_total_samples)


if __name__ == "__main__":
    pass
